# revision 16
# baseline (speedup 1.0000x reference)
"""Trainium2 Bass kernel for nn_AttentionLayer (attention pooling).

Reference computation (per sample b):
    scores[s, d] = tanh( sum_t X[t, d] * W[t, s] + bias[s] )   # X = inputs[b], [T=200, D=512]
    a = softmax over s of scores                                # per d column
    out[b, d] = sum_s a[s, d] * X[s, d]

Sharding: pure data parallel, batch 512 -> 64 samples on each of 8 cores.
W/b replicated. No collectives.

Kernel layout (per sample): s on partitions (chunks 128+72), d on free axis.
  - 4 float32r matmuls (W chunks stationary) accumulate scores into PSUM
  - ACT tanh (bias = b per partition) then ACT exp (bf16 out)
  - DVE multiply P = E * X (bf16 out)
  - bf16 ones-vector matmuls reduce over s -> numer/denom [1, 512] PSUM rows
  - per-sample DVE reciprocal + multiply into a stacked [64, 512] output tile
"""

import sys

if "/opt/trn_rl_repo" not in sys.path:
    sys.path.insert(0, "/opt/trn_rl_repo")

import ml_dtypes
import numpy as np

import concourse.bass as bass
import concourse.tile as tile
from concourse import bacc, mybir
from concourse.bass_utils import run_bass_kernel_spmd

B, T, D = 512, 200, 512
N_CORES = 8
NS = B // N_CORES  # samples per core
T0 = 128           # first t/s chunk
T1 = T - T0        # 72

F32 = mybir.dt.float32
F32R = mybir.dt.float32r
BF16 = mybir.dt.bfloat16

_CACHE = {}


def _build(ns=NS):
    nc = bacc.Bacc("TRN2", target_bir_lowering=False, debug=False)

    x_ext = nc.declare_dram_parameter("x", [ns, T, D], F32R, isOutput=False)
    w_ext = nc.declare_dram_parameter("w", [T, T], F32R, isOutput=False)
    b_ext = nc.declare_dram_parameter("bia", [T, 1], F32, isOutput=False)
    onesb_ext = nc.declare_dram_parameter("onesb", [T0, 1], BF16, isOutput=False)
    out_ext = nc.declare_dram_parameter("out", [ns, D], F32, isOutput=True)

    with tile.TileContext(nc) as tc:
        with (
            tc.tile_pool(name="const", bufs=1) as cpool,
            tc.tile_pool(name="tanh", bufs=1) as tpool,
            tc.tile_pool(name="exp", bufs=3) as epool,
            tc.tile_pool(name="xb", bufs=10) as xbpool,
            tc.tile_pool(name="small", bufs=3) as apool,
            tc.tile_pool(name="xin", bufs=10) as xpool,
            tc.tile_pool(name="psum", bufs=2, space="PSUM") as ppool,
        ):
            # constants: weights, bias, ones
            w0 = cpool.tile([T0, T], F32R)  # W[0:128, :]   (t on partitions, s free)
            w1 = cpool.tile([T1, T], F32R)  # W[128:200, :]
            b0 = cpool.tile([T0, 1], F32)
            b1 = cpool.tile([T1, 1], F32)
            onesb = cpool.tile([T0, 1], BF16)
            nc.sync.dma_start(w0[:], w_ext[0:T0, :])
            nc.sync.dma_start(w1[:], w_ext[T0:T, :])
            nc.sync.dma_start(b0[:], b_ext[0:T0, :])
            nc.sync.dma_start(b1[:], b_ext[T0:T, :])
            nc.sync.dma_start(onesb[:], onesb_ext[:])

            GD = 4  # samples per activation group
            GR = 4  # samples per reduction/epilogue group (psum rows 0/32/64/96)

            def emit_reductions(ctx):
                eg0, eg1, xbs, base = ctx
                for gr in range(GD // GR):
                    # numer rows at partitions {0,32,64,96} of R1, denoms in R2
                    r1 = ppool.tile([128, D], F32, tag="r1")
                    r2 = ppool.tile([128, D], F32, tag="r2")
                    for k in range(GR):
                        j = gr * GR + k
                        e0 = eg0[:, j * D : (j + 1) * D]
                        e1 = eg1[:, j * D : (j + 1) * D]
                        xb0, xb1 = xbs[j]
                        # P = E * X (bf16, DVE 2x mode)
                        p0 = apool.tile([T0, D], BF16, tag="p0")
                        p1 = apool.tile([T1, D], BF16, tag="p1")
                        nc.vector.tensor_mul(p0[:], e0, xb0[:])
                        nc.vector.tensor_mul(p1[:], e1, xb1[:])

                        # numer[d] = sum_s P[s, d] -> R1 row 32k (col-tiled)
                        # denom[d] = sum_s E[s, d] -> R2 row 32k
                        tp = (0, 32 * k)
                        ro = 32 * k
                        nc.tensor.matmul(
                            out=r1[ro : ro + 1, :], lhsT=onesb[:], rhs=p0[:],
                            start=True, stop=False, tile_position=tp,
                        )
                        nc.tensor.matmul(
                            out=r1[ro : ro + 1, :], lhsT=onesb[0:T1, :], rhs=p1[:],
                            start=False, stop=True, tile_position=tp,
                        )
                        nc.tensor.matmul(
                            out=r2[ro : ro + 1, :], lhsT=onesb[:], rhs=e0,
                            start=True, stop=False, tile_position=tp,
                        )
                        nc.tensor.matmul(
                            out=r2[ro : ro + 1, :], lhsT=onesb[0:T1, :], rhs=e1,
                            start=False, stop=True, tile_position=tp,
                        )

                    # batched epilogue over rows {0,32,64,96} (between-rows
                    # lanes hold garbage; only the 4 real rows are DMA'd out)
                    nrows = 32 * (GR - 1) + 1
                    rcp = apool.tile([nrows, D], F32, tag="rcp")
                    nc.vector.reciprocal(rcp[:], r2[0:nrows, :])
                    og = apool.tile([nrows, D], F32, tag="og")
                    nc.vector.tensor_mul(og[:], r1[0:nrows, :], rcp[:])
                    s0 = base + gr * GR
                    nc.sync.dma_start(
                        out_ext[s0 : s0 + GR, :], og[0:nrows:32, :],
                    )

            prev = None
            for gd in range(ns // GD):
                base = gd * GD
                xts = []
                xbs = []
                for j in range(GD):
                    x0 = xpool.tile([T0, D], F32R, tag="x0")
                    x1 = xpool.tile([T1, D], F32R, tag="x1")
                    nc.sync.dma_start(x0[:], x_ext[base + j, 0:T0, :])
                    nc.sync.dma_start(x1[:], x_ext[base + j, T0:T, :])
                    xts.append((x0, x1))
                    # bf16 copies of X for the DVE multiplies (GpSimd casts)
                    xb0 = xbpool.tile([T0, D], BF16, tag="xb0")
                    xb1 = xbpool.tile([T1, D], BF16, tag="xb1")
                    nc.gpsimd.tensor_copy(xb0[:], x0[:].bitcast(F32))
                    nc.gpsimd.tensor_copy(xb1[:], x1[:].bitcast(F32))
                    xbs.append((xb0, xb1))

                tg0 = tpool.tile([T0, GD * D], F32, tag="tg0")
                tg1 = tpool.tile([T1, GD * D], F32, tag="tg1")
                eg0 = epool.tile([T0, GD * D], BF16, tag="eg0")
                eg1 = epool.tile([T1, GD * D], BF16, tag="eg1")

                for j in range(GD):
                    xs0, xs1 = (t[:] for t in xts[j])
                    # scores[s, d] = sum_t W[t, s] * X[t, d], s chunks 128 + 72
                    ps0 = ppool.tile([T0, D], F32, tag="ps0")
                    ps1 = ppool.tile([T1, D], F32, tag="ps1")
                    nc.tensor.matmul(
                        out=ps0[:], lhsT=w0[:, 0:T0], rhs=xs0,
                        start=True, stop=False,
                    )
                    nc.tensor.matmul(
                        out=ps0[:], lhsT=w1[:, 0:T0], rhs=xs1,
                        start=False, stop=True,
                    )
                    nc.tensor.matmul(
                        out=ps1[:], lhsT=w0[:, T0:T], rhs=xs0,
                        start=True, stop=False,
                    )
                    nc.tensor.matmul(
                        out=ps1[:], lhsT=w1[:, T0:T], rhs=xs1,
                        start=False, stop=True,
                    )

                    # tanh(scores + b) into the group tile
                    nc.scalar.activation(
                        out=tg0[:, j * D : (j + 1) * D], in_=ps0[:],
                        func=mybir.ActivationFunctionType.Tanh, bias=b0[:],
                    )
                    nc.scalar.activation(
                        out=tg1[:, j * D : (j + 1) * D], in_=ps1[:],
                        func=mybir.ActivationFunctionType.Tanh, bias=b1[:],
                    )

                # exp over the whole group (bf16 out)
                nc.scalar.activation(
                    out=eg0[:], in_=tg0[:], func=mybir.ActivationFunctionType.Exp,
                )
                nc.scalar.activation(
                    out=eg1[:], in_=tg1[:], func=mybir.ActivationFunctionType.Exp,
                )

                if prev is not None:
                    emit_reductions(prev)
                prev = (eg0, eg1, xbs, base)
            emit_reductions(prev)

    nc.compile()
    return nc


def _get_nc(ns=NS):
    if ns not in _CACHE:
        _CACHE[ns] = _build(ns)
    return _CACHE[ns]


def _run(inputs, W, b, trace=False, **trace_kw):
    x = np.ascontiguousarray(np.asarray(inputs, dtype=np.float32))
    w = np.ascontiguousarray(np.asarray(W, dtype=np.float32))
    bv = np.ascontiguousarray(np.asarray(b, dtype=np.float32)).reshape(T, 1)
    onesb = np.ones((T0, 1), dtype=ml_dtypes.bfloat16)

    nc = _get_nc()
    in_maps = [
        {
            "x": np.ascontiguousarray(x[c * NS : (c + 1) * NS]),
            "w": w,
            "bia": bv,
            "onesb": onesb,
        }
        for c in range(N_CORES)
    ]
    res = run_bass_kernel_spmd(
        nc, in_maps, core_ids=list(range(N_CORES)), trace=trace, **trace_kw
    )
    out = np.concatenate([res.results[c]["out"] for c in range(N_CORES)], axis=0)
    return out, res


def kernel(**inputs) -> np.ndarray:
    out, _ = _run(inputs["inputs"], inputs["W"], inputs["b"])
    return out


# revision 17
# speedup vs baseline: 1.1773x; 1.1773x over previous
"""Trainium2 Bass kernel for nn_AttentionLayer (attention pooling).

Reference computation (per sample b):
    scores[s, d] = tanh( sum_t X[t, d] * W[t, s] + bias[s] )   # X = inputs[b], [T=200, D=512]
    a = softmax over s of scores                                # per d column
    out[b, d] = sum_s a[s, d] * X[s, d]

Sharding: pure data parallel, batch 512 -> 64 samples on each of 8 cores.
W/b replicated. No collectives.

Kernel layout (per sample): s on partitions (chunks 128+72), d on free axis.
  - 4 float32r matmuls (W chunks stationary) accumulate scores into PSUM
  - ACT tanh (bias = b per partition) then ACT exp (bf16 out)
  - DVE multiply P = E * X (bf16 out)
  - bf16 ones-vector matmuls reduce over s -> numer/denom [1, 512] PSUM rows
  - per-sample DVE reciprocal + multiply into a stacked [64, 512] output tile
"""

import sys

if "/opt/trn_rl_repo" not in sys.path:
    sys.path.insert(0, "/opt/trn_rl_repo")

import ml_dtypes
import numpy as np

import concourse.bass as bass
import concourse.tile as tile
from concourse import bacc, mybir
from concourse.bass_utils import run_bass_kernel_spmd

B, T, D = 512, 200, 512
N_CORES = 8
NS = B // N_CORES  # samples per core
T0 = 128           # first t/s chunk
T1 = T - T0        # 72

F32 = mybir.dt.float32
F32R = mybir.dt.float32r
BF16 = mybir.dt.bfloat16

_CACHE = {}


def _build(ns=NS):
    nc = bacc.Bacc("TRN2", target_bir_lowering=False, debug=False)

    x_ext = nc.declare_dram_parameter("x", [ns, T, D], F32R, isOutput=False)
    w_ext = nc.declare_dram_parameter("w", [T, T], F32R, isOutput=False)
    b_ext = nc.declare_dram_parameter("bia", [T, 1], F32, isOutput=False)
    onesb_ext = nc.declare_dram_parameter("onesb", [T0, 1], BF16, isOutput=False)
    out_ext = nc.declare_dram_parameter("out", [ns, D], F32, isOutput=True)

    with tile.TileContext(nc) as tc:
        with (
            tc.tile_pool(name="const", bufs=1) as cpool,
            tc.tile_pool(name="tanh", bufs=1) as tpool,
            tc.tile_pool(name="exp", bufs=3) as epool,
            tc.tile_pool(name="small", bufs=3) as apool,
            tc.tile_pool(name="xin", bufs=12) as xpool,
            tc.tile_pool(name="psum", bufs=2, space="PSUM") as ppool,
        ):
            # constants: weights, bias, ones
            w0 = cpool.tile([T0, T], F32R)  # W[0:128, :]   (t on partitions, s free)
            w1 = cpool.tile([T1, T], F32R)  # W[128:200, :]
            b0 = cpool.tile([T0, 1], F32)
            b1 = cpool.tile([T1, 1], F32)
            onesb = cpool.tile([T0, 1], BF16)
            nc.sync.dma_start(w0[:], w_ext[0:T0, :])
            nc.sync.dma_start(w1[:], w_ext[T0:T, :])
            nc.sync.dma_start(b0[:], b_ext[0:T0, :])
            nc.sync.dma_start(b1[:], b_ext[T0:T, :])
            nc.sync.dma_start(onesb[:], onesb_ext[:])

            GD = 4  # samples per activation group
            GR = 4  # samples per reduction/epilogue group (psum rows 0/32/64/96)

            def emit_reductions(ctx):
                eg0, eg1, xts, base = ctx
                for gr in range(GD // GR):
                    # numer rows at partitions {0,32,64,96} of R1, denoms in R2
                    r1 = ppool.tile([128, D], F32, tag="r1")
                    r2 = ppool.tile([128, D], F32, tag="r2")
                    for k in range(GR):
                        j = gr * GR + k
                        e0 = eg0[:, j * D : (j + 1) * D]
                        e1 = eg1[:, j * D : (j + 1) * D]
                        # P = E * X (bf16 out)
                        p0 = apool.tile([T0, D], BF16, tag="p0")
                        p1 = apool.tile([T1, D], BF16, tag="p1")
                        nc.vector.tensor_mul(p0[:], e0, xts[j][0][:].bitcast(F32))
                        nc.vector.tensor_mul(p1[:], e1, xts[j][1][:].bitcast(F32))

                        # numer[d] = sum_s P[s, d] -> R1 row 32k (col-tiled)
                        # denom[d] = sum_s E[s, d] -> R2 row 32k
                        tp = (0, 32 * k)
                        ro = 32 * k
                        nc.tensor.matmul(
                            out=r1[ro : ro + 1, :], lhsT=onesb[:], rhs=p0[:],
                            start=True, stop=False, tile_position=tp,
                        )
                        nc.tensor.matmul(
                            out=r1[ro : ro + 1, :], lhsT=onesb[0:T1, :], rhs=p1[:],
                            start=False, stop=True, tile_position=tp,
                        )
                        nc.tensor.matmul(
                            out=r2[ro : ro + 1, :], lhsT=onesb[:], rhs=e0,
                            start=True, stop=False, tile_position=tp,
                        )
                        nc.tensor.matmul(
                            out=r2[ro : ro + 1, :], lhsT=onesb[0:T1, :], rhs=e1,
                            start=False, stop=True, tile_position=tp,
                        )

                    # batched epilogue over rows {0,32,64,96} (between-rows
                    # lanes hold garbage; only the 4 real rows are DMA'd out)
                    nrows = 32 * (GR - 1) + 1
                    rcp = apool.tile([nrows, D], F32, tag="rcp")
                    nc.vector.reciprocal(rcp[:], r2[0:nrows, :])
                    og = apool.tile([nrows, D], F32, tag="og")
                    nc.vector.tensor_mul(og[:], r1[0:nrows, :], rcp[:])
                    s0 = base + gr * GR
                    nc.sync.dma_start(
                        out_ext[s0 : s0 + GR, :], og[0:nrows:32, :],
                    )

            prev = None
            for gd in range(ns // GD):
                base = gd * GD
                xts = []
                for j in range(GD):
                    x0 = xpool.tile([T0, D], F32R, tag="x0")
                    x1 = xpool.tile([T1, D], F32R, tag="x1")
                    nc.sync.dma_start(x0[:], x_ext[base + j, 0:T0, :])
                    nc.sync.dma_start(x1[:], x_ext[base + j, T0:T, :])
                    xts.append((x0, x1))

                tg0 = tpool.tile([T0, GD * D], F32, tag="tg0")
                tg1 = tpool.tile([T1, GD * D], F32, tag="tg1")
                eg0 = epool.tile([T0, GD * D], BF16, tag="eg0")
                eg1 = epool.tile([T1, GD * D], BF16, tag="eg1")

                for j in range(GD):
                    xs0, xs1 = (t[:] for t in xts[j])
                    # scores[s, d] = sum_t W[t, s] * X[t, d], s chunks 128 + 72
                    ps0 = ppool.tile([T0, D], F32, tag="ps0")
                    ps1 = ppool.tile([T1, D], F32, tag="ps1")
                    nc.tensor.matmul(
                        out=ps0[:], lhsT=w0[:, 0:T0], rhs=xs0,
                        start=True, stop=False,
                    )
                    nc.tensor.matmul(
                        out=ps0[:], lhsT=w1[:, 0:T0], rhs=xs1,
                        start=False, stop=True,
                    )
                    nc.tensor.matmul(
                        out=ps1[:], lhsT=w0[:, T0:T], rhs=xs0,
                        start=True, stop=False,
                    )
                    nc.tensor.matmul(
                        out=ps1[:], lhsT=w1[:, T0:T], rhs=xs1,
                        start=False, stop=True,
                    )

                    # tanh(scores + b) into the group tile
                    nc.scalar.activation(
                        out=tg0[:, j * D : (j + 1) * D], in_=ps0[:],
                        func=mybir.ActivationFunctionType.Tanh, bias=b0[:],
                    )
                    nc.scalar.activation(
                        out=tg1[:, j * D : (j + 1) * D], in_=ps1[:],
                        func=mybir.ActivationFunctionType.Tanh, bias=b1[:],
                    )

                # exp over the whole group (bf16 out)
                nc.scalar.activation(
                    out=eg0[:], in_=tg0[:], func=mybir.ActivationFunctionType.Exp,
                )
                nc.scalar.activation(
                    out=eg1[:], in_=tg1[:], func=mybir.ActivationFunctionType.Exp,
                )

                if prev is not None:
                    emit_reductions(prev)
                prev = (eg0, eg1, xts, base)
            emit_reductions(prev)

    nc.compile()
    return nc


def _get_nc(ns=NS):
    if ns not in _CACHE:
        _CACHE[ns] = _build(ns)
    return _CACHE[ns]


def _run(inputs, W, b, trace=False, **trace_kw):
    x = np.ascontiguousarray(np.asarray(inputs, dtype=np.float32))
    w = np.ascontiguousarray(np.asarray(W, dtype=np.float32))
    bv = np.ascontiguousarray(np.asarray(b, dtype=np.float32)).reshape(T, 1)
    onesb = np.ones((T0, 1), dtype=ml_dtypes.bfloat16)

    nc = _get_nc()
    in_maps = [
        {
            "x": np.ascontiguousarray(x[c * NS : (c + 1) * NS]),
            "w": w,
            "bia": bv,
            "onesb": onesb,
        }
        for c in range(N_CORES)
    ]
    res = run_bass_kernel_spmd(
        nc, in_maps, core_ids=list(range(N_CORES)), trace=trace, **trace_kw
    )
    out = np.concatenate([res.results[c]["out"] for c in range(N_CORES)], axis=0)
    return out, res


def kernel(**inputs) -> np.ndarray:
    out, _ = _run(inputs["inputs"], inputs["W"], inputs["b"])
    return out


# revision 18
# speedup vs baseline: 1.4540x; 1.2350x over previous
"""Trainium2 Bass kernel for nn_AttentionLayer (attention pooling).

Reference computation (per sample b):
    scores[s, d] = tanh( sum_t X[t, d] * W[t, s] + bias[s] )   # X = inputs[b], [T=200, D=512]
    a = softmax over s of scores                                # per d column
    out[b, d] = sum_s a[s, d] * X[s, d]

Sharding: pure data parallel, batch 512 -> 64 samples on each of 8 cores.
W/b replicated. No collectives.

Kernel layout (per sample): s on partitions (chunks 128+72), d on free axis.
  - 4 float32r matmuls (W chunks stationary) accumulate scores into PSUM
  - ACT tanh (bias = b per partition) then ACT exp (bf16 out)
  - DVE multiply P = E * X (bf16 out)
  - bf16 ones-vector matmuls reduce over s -> numer/denom [1, 512] PSUM rows
  - per-sample DVE reciprocal + multiply into a stacked [64, 512] output tile
"""

import sys

if "/opt/trn_rl_repo" not in sys.path:
    sys.path.insert(0, "/opt/trn_rl_repo")

import ml_dtypes
import numpy as np

import concourse.bass as bass
import concourse.tile as tile
from concourse import bacc, mybir
from concourse.bass_utils import run_bass_kernel_spmd

B, T, D = 512, 200, 512
N_CORES = 8
NS = B // N_CORES  # samples per core
T0 = 128           # first t/s chunk
T1 = T - T0        # 72

F32 = mybir.dt.float32
F32R = mybir.dt.float32r
BF16 = mybir.dt.bfloat16

_CACHE = {}


def _build(ns=NS):
    nc = bacc.Bacc("TRN2", target_bir_lowering=False, debug=False)

    x_ext = nc.declare_dram_parameter("x", [ns, T, D], F32R, isOutput=False)
    w_ext = nc.declare_dram_parameter("w", [T, T], F32R, isOutput=False)
    b_ext = nc.declare_dram_parameter("bia", [T, 1], F32, isOutput=False)
    onesb_ext = nc.declare_dram_parameter("onesb", [T0, 1], BF16, isOutput=False)
    out_ext = nc.declare_dram_parameter("out", [ns, D], F32, isOutput=True)

    with tile.TileContext(nc) as tc:
        with (
            tc.tile_pool(name="const", bufs=1) as cpool,
            tc.tile_pool(name="tanh", bufs=2) as tpool,
            tc.tile_pool(name="exp", bufs=2) as epool,
            tc.tile_pool(name="small", bufs=3) as apool,
            tc.tile_pool(name="xin", bufs=3) as xpool,
            tc.tile_pool(name="psum", bufs=2, space="PSUM") as ppool,
        ):
            # constants: weights, bias, ones
            w0 = cpool.tile([T0, T], F32R)  # W[0:128, :]   (t on partitions, s free)
            w1 = cpool.tile([T1, T], F32R)  # W[128:200, :]
            b0 = cpool.tile([T0, 1], F32)
            b1 = cpool.tile([T1, 1], F32)
            onesb = cpool.tile([T0, 1], BF16)
            nc.sync.dma_start(w0[:], w_ext[0:T0, :])
            nc.sync.dma_start(w1[:], w_ext[T0:T, :])
            nc.sync.dma_start(b0[:], b_ext[0:T0, :])
            nc.sync.dma_start(b1[:], b_ext[T0:T, :])
            nc.sync.dma_start(onesb[:], onesb_ext[:])

            GD = 4  # samples per group (activation batching + psum rows 0/32/64/96)
            for gd in range(ns // GD):
                base = gd * GD
                # X for the group: per-sample DMAs (queue parallelism) into
                # one group tile per t-chunk
                xg0 = xpool.tile([T0, GD * D], F32R, tag="xg0")
                xg1 = xpool.tile([T1, GD * D], F32R, tag="xg1")
                for j in range(GD):
                    nc.sync.dma_start(
                        xg0[:, j * D : (j + 1) * D], x_ext[base + j, 0:T0, :]
                    )
                    nc.sync.dma_start(
                        xg1[:, j * D : (j + 1) * D], x_ext[base + j, T0:T, :]
                    )

                tg0 = tpool.tile([T0, GD * D], F32, tag="tg0")
                tg1 = tpool.tile([T1, GD * D], F32, tag="tg1")
                eg0 = epool.tile([T0, GD * D], BF16, tag="eg0")
                eg1 = epool.tile([T1, GD * D], BF16, tag="eg1")

                for j in range(GD):
                    xs0 = xg0[:, j * D : (j + 1) * D]
                    xs1 = xg1[:, j * D : (j + 1) * D]
                    # scores[s, d] = sum_t W[t, s] * X[t, d], s chunks 128 + 72
                    ps0 = ppool.tile([T0, D], F32, tag="ps0")
                    ps1 = ppool.tile([T1, D], F32, tag="ps1")
                    nc.tensor.matmul(
                        out=ps0[:], lhsT=w0[:, 0:T0], rhs=xs0,
                        start=True, stop=False,
                    )
                    nc.tensor.matmul(
                        out=ps0[:], lhsT=w1[:, 0:T0], rhs=xs1,
                        start=False, stop=True,
                    )
                    nc.tensor.matmul(
                        out=ps1[:], lhsT=w0[:, T0:T], rhs=xs0,
                        start=True, stop=False,
                    )
                    nc.tensor.matmul(
                        out=ps1[:], lhsT=w1[:, T0:T], rhs=xs1,
                        start=False, stop=True,
                    )

                    # tanh(scores + b) into the group tile
                    nc.scalar.activation(
                        out=tg0[:, j * D : (j + 1) * D], in_=ps0[:],
                        func=mybir.ActivationFunctionType.Tanh, bias=b0[:],
                    )
                    nc.scalar.activation(
                        out=tg1[:, j * D : (j + 1) * D], in_=ps1[:],
                        func=mybir.ActivationFunctionType.Tanh, bias=b1[:],
                    )

                # exp over the whole group (bf16 out)
                nc.scalar.activation(
                    out=eg0[:], in_=tg0[:], func=mybir.ActivationFunctionType.Exp,
                )
                nc.scalar.activation(
                    out=eg1[:], in_=tg1[:], func=mybir.ActivationFunctionType.Exp,
                )

                # P = E * X for the whole group (bf16 out, one DVE op per chunk)
                pg0 = epool.tile([T0, GD * D], BF16, tag="pg0")
                pg1 = epool.tile([T1, GD * D], BF16, tag="pg1")
                nc.vector.tensor_mul(pg0[:], eg0[:], xg0[:].bitcast(F32))
                nc.vector.tensor_mul(pg1[:], eg1[:], xg1[:].bitcast(F32))

                # numer rows at partitions {0,32,64,96} of R1, denoms in R2
                r1 = ppool.tile([128, D], F32, tag="r1")
                r2 = ppool.tile([128, D], F32, tag="r2")
                for k in range(GD):
                    e0 = eg0[:, k * D : (k + 1) * D]
                    e1 = eg1[:, k * D : (k + 1) * D]
                    p0 = pg0[:, k * D : (k + 1) * D]
                    p1 = pg1[:, k * D : (k + 1) * D]
                    # numer[d] = sum_s P[s, d] -> R1 row 32k (col-tiled)
                    # denom[d] = sum_s E[s, d] -> R2 row 32k
                    tp = (0, 32 * k)
                    ro = 32 * k
                    nc.tensor.matmul(
                        out=r1[ro : ro + 1, :], lhsT=onesb[:], rhs=p0,
                        start=True, stop=False, tile_position=tp,
                    )
                    nc.tensor.matmul(
                        out=r1[ro : ro + 1, :], lhsT=onesb[0:T1, :], rhs=p1,
                        start=False, stop=True, tile_position=tp,
                    )
                    nc.tensor.matmul(
                        out=r2[ro : ro + 1, :], lhsT=onesb[:], rhs=e0,
                        start=True, stop=False, tile_position=tp,
                    )
                    nc.tensor.matmul(
                        out=r2[ro : ro + 1, :], lhsT=onesb[0:T1, :], rhs=e1,
                        start=False, stop=True, tile_position=tp,
                    )

                # batched epilogue over rows {0,32,64,96} (between-rows lanes
                # hold garbage; only the 4 real rows are DMA'd out)
                nrows = 32 * (GD - 1) + 1
                rcp = apool.tile([nrows, D], F32, tag="rcp")
                nc.vector.reciprocal(rcp[:], r2[0:nrows, :])
                og = apool.tile([nrows, D], F32, tag="og")
                nc.vector.tensor_mul(og[:], r1[0:nrows, :], rcp[:])
                nc.sync.dma_start(
                    out_ext[base : base + GD, :], og[0:nrows:32, :],
                )

    nc.compile()
    return nc


def _get_nc(ns=NS):
    if ns not in _CACHE:
        _CACHE[ns] = _build(ns)
    return _CACHE[ns]


def _run(inputs, W, b, trace=False, **trace_kw):
    x = np.ascontiguousarray(np.asarray(inputs, dtype=np.float32))
    w = np.ascontiguousarray(np.asarray(W, dtype=np.float32))
    bv = np.ascontiguousarray(np.asarray(b, dtype=np.float32)).reshape(T, 1)
    onesb = np.ones((T0, 1), dtype=ml_dtypes.bfloat16)

    nc = _get_nc()
    in_maps = [
        {
            "x": np.ascontiguousarray(x[c * NS : (c + 1) * NS]),
            "w": w,
            "bia": bv,
            "onesb": onesb,
        }
        for c in range(N_CORES)
    ]
    res = run_bass_kernel_spmd(
        nc, in_maps, core_ids=list(range(N_CORES)), trace=trace, **trace_kw
    )
    out = np.concatenate([res.results[c]["out"] for c in range(N_CORES)], axis=0)
    return out, res


def kernel(**inputs) -> np.ndarray:
    out, _ = _run(inputs["inputs"], inputs["W"], inputs["b"])
    return out
